# revision 1
# baseline (speedup 1.0000x reference)
"""HGNN conv on 8 trn2 cores.

out = D_v^-1 H D_e^-1 H^T input W + bias   (W commutes past the aggregations)

Phase A (edge-sharded): cores own contiguous 3200-edge ranges. Entries sorted
by E; per 128-edge window, entry tiles are gathered (indirect DMA on V) and
scatter-accumulated into PSUM via one-hot matmul; window * recip_e -> y_shard.
AllGather y_shard -> y_full [25600,128] on every core.
Phase B (node-sharded): cores own contiguous 6272-node ranges. Entries sorted
by V; per 128-node window, gather y_full rows by E, one-hot matmul with
swapped operands accumulates Z^T; then Z@W (* recip_v) + bias -> out rows.
"""
import os
import sys

for _p in ('/opt/trn_rl_repo', '/root/.axon_site/_ro/trn_rl_repo'):
    if os.path.isdir(_p) and _p not in sys.path:
        sys.path.insert(0, _p)

import numpy as np

P = 128
NCORES = 8
N_NODE = 50000
N_EDGE = 25000
D = 128
ESH = 3200            # edges per core shard (8*3200 = 25600 >= 25000)
NSH = 6272            # nodes per core shard (49*128; 8*6272 = 50176 >= 50000)
W_A = ESH // P        # 25 edge windows per core
W_B = NSH // P        # 49 node windows per core

_PROG_CACHE = {}
LAST_RESULTS = None


def _pack_windows(sorted_idx, sorted_slotbase, bnd, n_shards, n_win, F):
    """sorted_idx: gather row per entry; bnd: window entry boundaries."""
    vidx = np.zeros((n_shards, n_win, P, F), np.int32)
    slot = np.full((n_shards, n_win, P, F), -1.0, np.float32)
    for c in range(n_shards):
        for w in range(n_win):
            wi = c * n_win + w
            lo, hi = bnd[wi], bnd[wi + 1]
            n = hi - lo
            if n == 0:
                continue
            vv = np.zeros(F * P, np.int32)
            vv[:n] = sorted_idx[lo:hi]
            ss = np.full(F * P, -1.0, np.float32)
            ss[:n] = sorted_slotbase[lo:hi] - wi * P
            vidx[c, w] = vv.reshape(F, P).T
            slot[c, w] = ss.reshape(F, P).T
    return vidx, slot


def _preprocess(V, E):
    V = np.asarray(V).astype(np.int64)
    E = np.asarray(E).astype(np.int64)

    oA = np.argsort(E, kind='stable')
    Es, Vs = E[oA], V[oA]
    bndA = np.searchsorted(Es, np.arange(0, NCORES * ESH + 1, P))
    FA = int(np.ceil(np.diff(bndA).max() / P))
    a_vidx, a_slot = _pack_windows(Vs, Es, bndA, NCORES, W_A, FA)
    cntE = np.bincount(E, minlength=N_EDGE).astype(np.float64)
    recipE = (1.0 / np.maximum(cntE, 1.0)).astype(np.float32)
    er = np.arange(NCORES * ESH)
    a_recip = np.where(er < N_EDGE, recipE[np.minimum(er, N_EDGE - 1)],
                       0.0).astype(np.float32).reshape(NCORES, W_A, P)

    oB = np.argsort(V, kind='stable')
    Vs2, Es2 = V[oB], E[oB]
    bndB = np.searchsorted(Vs2, np.arange(0, NCORES * NSH + 1, P))
    FB = int(np.ceil(np.diff(bndB).max() / P))
    b_eidx, b_slot = _pack_windows(Es2, Vs2, bndB, NCORES, W_B, FB)
    cntV = np.bincount(V, minlength=N_NODE).astype(np.float64)
    recipV = (1.0 / np.maximum(cntV, 1.0)).astype(np.float32)
    nr = np.arange(NCORES * NSH)
    b_recip = np.where(nr < N_NODE, recipV[np.minimum(nr, N_NODE - 1)],
                       0.0).astype(np.float32).reshape(NCORES, W_B, P)

    return dict(FA=FA, FB=FB, a_vidx=a_vidx, a_slot=a_slot, a_recip=a_recip,
                b_eidx=b_eidx, b_slot=b_slot, b_recip=b_recip)


def _emulate(pp, inp_f32, weight, bias):
    """Numpy emulation of the exact device program (for logic validation)."""
    FA, FB = pp['FA'], pp['FB']
    iota = np.arange(P, dtype=np.float32)
    y_full = np.zeros((NCORES * ESH, D), np.float32)
    for c in range(NCORES):
        for w in range(W_A):
            acc = np.zeros((P, D), np.float32)
            for f in range(FA):
                g = inp_f32[pp['a_vidx'][c, w, :, f]]
                o = (iota[None, :] == pp['a_slot'][c, w, :, f][:, None])
                acc += o.astype(np.float32).T @ g
            y_full[(c * W_A + w) * P:(c * W_A + w + 1) * P] = \
                acc * pp['a_recip'][c, w][:, None]
    out = np.zeros((NCORES * NSH, D), np.float32)
    for c in range(NCORES):
        for w in range(W_B):
            acc2 = np.zeros((D, P), np.float32)
            for f in range(FB):
                g = y_full[pp['b_eidx'][c, w, :, f]]
                o = (iota[None, :] == pp['b_slot'][c, w, :, f][:, None])
                acc2 += g.T @ o.astype(np.float32)
            z = acc2.T
            res = (z @ weight) * pp['b_recip'][c, w][:, None] + bias[None, :]
            out[(c * W_B + w) * P + c * (NSH - W_B * P):][:P] = res  # NSH==W_B*P
    rows = []
    for c in range(NCORES):
        n = min(NSH, N_NODE - c * NSH)
        rows.append(out[c * NSH:c * NSH + n])
    return np.concatenate(rows, 0)


def _build_program(FA, FB):
    import concourse.bacc as bacc
    import concourse.bass as bass
    import concourse.tile as tile
    from concourse import mybir

    f32 = mybir.dt.float32
    i32 = mybir.dt.int32

    nc = bacc.Bacc(None, target_bir_lowering=False, debug=False)
    inp = nc.dram_tensor("input", [N_NODE, D], f32, kind="ExternalInput")
    wgt = nc.dram_tensor("wgt", [D, D], f32, kind="ExternalInput")
    bias_bc = nc.dram_tensor("bias_bc", [P, D], f32, kind="ExternalInput")
    iota_in = nc.dram_tensor("iota_in", [P, P], f32, kind="ExternalInput")
    a_vidx = nc.dram_tensor("a_vidx", [W_A, P, FA], i32, kind="ExternalInput")
    a_slot = nc.dram_tensor("a_slot", [W_A, P, FA], f32, kind="ExternalInput")
    a_recip = nc.dram_tensor("a_recip", [W_A, P], f32, kind="ExternalInput")
    b_eidx = nc.dram_tensor("b_eidx", [W_B, P, FB], i32, kind="ExternalInput")
    b_slot = nc.dram_tensor("b_slot", [W_B, P, FB], f32, kind="ExternalInput")
    b_recip = nc.dram_tensor("b_recip", [W_B, P], f32, kind="ExternalInput")
    out = nc.dram_tensor("out", [NSH, D], f32, kind="ExternalOutput")

    eq = mybir.AluOpType.is_equal
    mul = mybir.AluOpType.mult
    add = mybir.AluOpType.add

    with tile.TileContext(nc) as tc:
        with tc.tile_pool(name="const", bufs=1) as cpool, \
             tc.tile_pool(name="idx", bufs=3) as ipool, \
             tc.tile_pool(name="slt", bufs=3) as spool, \
             tc.tile_pool(name="rcp", bufs=3) as rpool, \
             tc.tile_pool(name="gat", bufs=12) as gpool, \
             tc.tile_pool(name="one", bufs=12) as opool, \
             tc.tile_pool(name="res", bufs=4) as respool, \
             tc.tile_pool(name="pacc", bufs=2, space="PSUM") as pacc, \
             tc.tile_pool(name="pres", bufs=2, space="PSUM") as pres, \
             tc.tile_pool(name="dram", bufs=1, space="DRAM") as dpool:

            iota_t = cpool.tile([P, P], f32)
            nc.sync.dma_start(out=iota_t[:], in_=iota_in[:])
            wgt_t = cpool.tile([D, D], f32)
            nc.sync.dma_start(out=wgt_t[:], in_=wgt[:])
            bias_t = cpool.tile([P, D], f32)
            nc.sync.dma_start(out=bias_t[:], in_=bias_bc[:])

            y_shard = dpool.tile([ESH, D], f32)
            y_full = dpool.tile([NCORES * ESH, D], f32, addr_space="Shared")

            # ---------------- Phase A ----------------
            for w in range(W_A):
                vidx_t = ipool.tile([P, FA], i32)
                nc.sync.dma_start(out=vidx_t[:], in_=a_vidx[w])
                slot_t = spool.tile([P, FA], f32)
                nc.sync.dma_start(out=slot_t[:], in_=a_slot[w])
                recip_t = rpool.tile([P, 1], f32)
                nc.sync.dma_start(out=recip_t[:], in_=a_recip[w, :, None])
                acc = pacc.tile([P, D], f32)
                for f in range(FA):
                    g = gpool.tile([P, D], f32)
                    nc.gpsimd.indirect_dma_start(
                        out=g[:], out_offset=None, in_=inp[:],
                        in_offset=bass.IndirectOffsetOnAxis(
                            ap=vidx_t[:, f:f + 1], axis=0))
                    o = opool.tile([P, P], f32)
                    nc.vector.tensor_tensor(
                        out=o[:], in0=iota_t[:],
                        in1=slot_t[:, f:f + 1].to_broadcast([P, P]), op=eq)
                    nc.tensor.matmul(acc[:], lhsT=o[:], rhs=g[:],
                                     start=(f == 0), stop=(f == FA - 1))
                yw = respool.tile([P, D], f32)
                nc.vector.tensor_tensor(
                    out=yw[:], in0=acc[:],
                    in1=recip_t[:, :1].to_broadcast([P, D]), op=mul)
                nc.sync.dma_start(out=y_shard[w * P:(w + 1) * P, :], in_=yw[:])

            nc.gpsimd.collective_compute(
                "AllGather", mybir.AluOpType.bypass,
                replica_groups=[list(range(NCORES))],
                ins=[y_shard.opt()], outs=[y_full.opt()])

            # ---------------- Phase B ----------------
            for w in range(W_B):
                eidx_t = ipool.tile([P, FB], i32, name="eidx_t", tag="idx_b")
                nc.sync.dma_start(out=eidx_t[:], in_=b_eidx[w])
                slot_t = spool.tile([P, FB], f32, name="slot_tb", tag="slt_b")
                nc.sync.dma_start(out=slot_t[:], in_=b_slot[w])
                recip_t = rpool.tile([P, 1], f32, name="recip_tb")
                nc.sync.dma_start(out=recip_t[:], in_=b_recip[w, :, None])
                acc2 = pacc.tile([P, D], f32, name="acc2")
                for f in range(FB):
                    g = gpool.tile([P, D], f32, name="gb")
                    nc.gpsimd.indirect_dma_start(
                        out=g[:], out_offset=None, in_=y_full[:],
                        in_offset=bass.IndirectOffsetOnAxis(
                            ap=eidx_t[:, f:f + 1], axis=0))
                    o = opool.tile([P, P], f32, name="ob")
                    nc.vector.tensor_tensor(
                        out=o[:], in0=iota_t[:],
                        in1=slot_t[:, f:f + 1].to_broadcast([P, P]), op=eq)
                    nc.tensor.matmul(acc2[:], lhsT=g[:], rhs=o[:],
                                     start=(f == 0), stop=(f == FB - 1))
                zt = respool.tile([P, D], f32, name="zt")
                nc.vector.tensor_copy(out=zt[:], in_=acc2[:])
                res_p = pres.tile([P, D], f32, name="res_p")
                nc.tensor.matmul(res_p[:], lhsT=zt[:], rhs=wgt_t[:],
                                 start=True, stop=True)
                tmp = respool.tile([P, D], f32, name="tmpb")
                nc.vector.tensor_tensor(
                    out=tmp[:], in0=res_p[:],
                    in1=recip_t[:, :1].to_broadcast([P, D]), op=mul)
                res = respool.tile([P, D], f32, name="resb")
                nc.vector.tensor_tensor(out=res[:], in0=tmp[:], in1=bias_t[:],
                                        op=add)
                nc.sync.dma_start(out=out[w * P:(w + 1) * P, :], in_=res[:])

    nc.compile()
    return nc


def kernel(input, weight, bias, V, E, num_edges):
    global LAST_RESULTS
    inp = np.ascontiguousarray(np.asarray(input), dtype=np.float32)
    wgt = np.ascontiguousarray(np.asarray(weight), dtype=np.float32)
    b = np.asarray(bias).astype(np.float32)
    pp = _preprocess(V, E)

    if os.environ.get('KERNEL_EMULATE'):
        return _emulate(pp, inp, wgt, b)

    from concourse.bass_utils import run_bass_kernel_spmd

    key = (pp['FA'], pp['FB'])
    if key not in _PROG_CACHE:
        _PROG_CACHE[key] = _build_program(*key)
    nc = _PROG_CACHE[key]

    iota_np = np.tile(np.arange(P, dtype=np.float32), (P, 1))
    bias_bc = np.tile(b[None, :], (P, 1)).astype(np.float32)
    in_maps = []
    for c in range(NCORES):
        in_maps.append(dict(
            input=inp, wgt=wgt, bias_bc=bias_bc, iota_in=iota_np,
            a_vidx=pp['a_vidx'][c], a_slot=pp['a_slot'][c],
            a_recip=pp['a_recip'][c],
            b_eidx=pp['b_eidx'][c], b_slot=pp['b_slot'][c],
            b_recip=pp['b_recip'][c]))

    trace = bool(os.environ.get('KERNEL_TRACE'))
    res = run_bass_kernel_spmd(nc, in_maps, list(range(NCORES)), trace=trace)
    LAST_RESULTS = res
    rows = []
    for c in range(NCORES):
        n = min(NSH, N_NODE - c * NSH)
        rows.append(res.results[c]['out'][:n])
    return np.concatenate(rows, 0).astype(np.float32)


# revision 11
# speedup vs baseline: 1.5927x; 1.5927x over previous
"""HGNN conv on 8 trn2 cores.

out = D_v^-1 H D_e^-1 H^T input W + bias   (W commutes past the aggregations)

Phase A (edge-sharded): cores own contiguous 3200-edge ranges. Entries sorted
by E; per 128-edge window, entry tiles are gathered (indirect DMA on V) and
scatter-accumulated into PSUM via one-hot matmul; window * recip_e -> y_shard.
AllGather y_shard -> y_full [25600,128] on every core.
Phase B (node-sharded): cores own contiguous 6272-node ranges. Entries sorted
by V; per 128-node window, gather y_full rows by E, one-hot matmul with
swapped operands accumulates Z^T; then Z@W (* recip_v) + bias -> out rows.
"""
import os
import sys

for _p in ('/opt/trn_rl_repo', '/root/.axon_site/_ro/trn_rl_repo'):
    if os.path.isdir(_p) and _p not in sys.path:
        sys.path.insert(0, _p)

import numpy as np

P = 128
NCORES = 8
N_NODE = 50000
N_EDGE = 25000
D = 128
ESH = 3200            # edges per core shard (8*3200 = 25600 >= 25000)
NSH = 6272            # nodes per core shard (49*128; 8*6272 = 50176 >= 50000)
W_A = ESH // P        # 25 edge windows per core
W_B = NSH // P        # 49 node windows per core
NCH = 5               # allgather chunks (5 windows = 640 edges each)
CH_E = ESH // NCH     # 640 edges per chunk per core


def _row_of_edge(e):
    """y_full row for global edge id, chunk-major allgather layout."""
    c = e // ESH
    k = (e % ESH) // CH_E
    return k * (NCORES * CH_E) + c * CH_E + e % CH_E

_PROG_CACHE = {}
LAST_RESULTS = None


def _pack_windows(sorted_idx, sorted_slotbase, bnd, n_shards, n_win, F):
    """sorted_idx: gather row per entry; bnd: window entry boundaries."""
    vidx = np.zeros((n_shards, n_win, P, F), np.int32)
    slot = np.full((n_shards, n_win, P, F), -1.0, np.float32)
    for c in range(n_shards):
        for w in range(n_win):
            wi = c * n_win + w
            lo, hi = bnd[wi], bnd[wi + 1]
            n = hi - lo
            if n == 0:
                continue
            vv = np.zeros(F * P, np.int32)
            vv[:n] = sorted_idx[lo:hi]
            ss = np.full(F * P, -1.0, np.float32)
            ss[:n] = sorted_slotbase[lo:hi] - wi * P
            vidx[c, w] = vv.reshape(F, P).T
            slot[c, w] = ss.reshape(F, P).T
    return vidx, slot


def _preprocess(V, E):
    V = np.asarray(V).astype(np.int64)
    E = np.asarray(E).astype(np.int64)

    oA = np.argsort(E, kind='stable')
    Es, Vs = E[oA], V[oA]
    bndA = np.searchsorted(Es, np.arange(0, NCORES * ESH + 1, P))
    FA = int(np.ceil(np.diff(bndA).max() / P))
    a_vidx, a_slot = _pack_windows(Vs, Es, bndA, NCORES, W_A, FA)
    cntE = np.bincount(E, minlength=N_EDGE).astype(np.float64)
    recipE = (1.0 / np.maximum(cntE, 1.0)).astype(np.float32)
    er = np.arange(NCORES * ESH)
    a_recip = np.where(er < N_EDGE, recipE[np.minimum(er, N_EDGE - 1)],
                       0.0).astype(np.float32).reshape(NCORES, W_A, P)

    oB = np.argsort(V, kind='stable')
    Vs2, Es2 = V[oB], E[oB]
    bndB = np.searchsorted(Vs2, np.arange(0, NCORES * NSH + 1, P))
    FB = int(np.ceil(np.diff(bndB).max() / P))
    b_eidx, b_slot = _pack_windows(_row_of_edge(Es2), Vs2, bndB,
                                   NCORES, W_B, FB)
    cntV = np.bincount(V, minlength=N_NODE).astype(np.float64)
    recipV = (1.0 / np.maximum(cntV, 1.0)).astype(np.float32)
    nr = np.arange(NCORES * NSH)
    b_recip = np.where(nr < N_NODE, recipV[np.minimum(nr, N_NODE - 1)],
                       0.0).astype(np.float32).reshape(NCORES, W_B, P)

    return dict(FA=FA, FB=FB, a_vidx=a_vidx, a_slot=a_slot, a_recip=a_recip,
                b_eidx=b_eidx, b_slot=b_slot, b_recip=b_recip)


def _emulate(pp, inp_f32, weight, bias):
    """Numpy emulation of the exact device program (for logic validation)."""
    FA, FB = pp['FA'], pp['FB']
    iota = np.arange(P, dtype=np.float32)
    y_full = np.zeros((NCORES * ESH, D), np.float32)
    for c in range(NCORES):
        for w in range(W_A):
            acc = np.zeros((P, D), np.float32)
            for f in range(FA):
                g = inp_f32[pp['a_vidx'][c, w, :, f]]
                o = (iota[None, :] == pp['a_slot'][c, w, :, f][:, None])
                acc += o.astype(np.float32).T @ g
            r0 = _row_of_edge(c * ESH + w * P)
            y_full[r0:r0 + P] = acc * pp['a_recip'][c, w][:, None]
    out = np.zeros((NCORES * NSH, D), np.float32)
    for c in range(NCORES):
        for w in range(W_B):
            acc2 = np.zeros((D, P), np.float32)
            for f in range(FB):
                g = y_full[pp['b_eidx'][c, w, :, f]]
                o = (iota[None, :] == pp['b_slot'][c, w, :, f][:, None])
                acc2 += g.T @ o.astype(np.float32)
            z = acc2.T
            res = (z @ weight) * pp['b_recip'][c, w][:, None] + bias[None, :]
            out[(c * W_B + w) * P + c * (NSH - W_B * P):][:P] = res  # NSH==W_B*P
    rows = []
    for c in range(NCORES):
        n = min(NSH, N_NODE - c * NSH)
        rows.append(out[c * NSH:c * NSH + n])
    return np.concatenate(rows, 0)


def _build_program(FA, FB):
    import concourse.bacc as bacc
    import concourse.bass as bass
    import concourse.tile as tile
    from concourse import mybir

    f32 = mybir.dt.float32
    bf16 = mybir.dt.bfloat16
    i32 = mybir.dt.int32

    nc = bacc.Bacc(None, target_bir_lowering=False, debug=False)
    inp = nc.dram_tensor("input", [N_NODE, D], bf16, kind="ExternalInput")
    wgt = nc.dram_tensor("wgt", [D, D], bf16, kind="ExternalInput")
    bias_bc = nc.dram_tensor("bias_bc", [P, D], f32, kind="ExternalInput")
    iota_in = nc.dram_tensor("iota_in", [P, P], f32, kind="ExternalInput")
    a_vidx = nc.dram_tensor("a_vidx", [W_A, P, FA], i32, kind="ExternalInput")
    a_slot = nc.dram_tensor("a_slot", [W_A, P, FA], f32, kind="ExternalInput")
    a_recip = nc.dram_tensor("a_recip", [W_A, P], f32, kind="ExternalInput")
    b_eidx = nc.dram_tensor("b_eidx", [W_B, P, FB], i32, kind="ExternalInput")
    b_slot = nc.dram_tensor("b_slot", [W_B, P, FB], f32, kind="ExternalInput")
    b_recip = nc.dram_tensor("b_recip", [W_B, P], f32, kind="ExternalInput")
    out = nc.dram_tensor("out", [NSH, D], f32, kind="ExternalOutput")

    eq = mybir.AluOpType.is_equal
    mul = mybir.AluOpType.mult
    add = mybir.AluOpType.add

    with tile.TileContext(nc) as tc:
        with tc.tile_pool(name="const", bufs=1) as cpool, \
             tc.tile_pool(name="idx", bufs=3) as ipool, \
             tc.tile_pool(name="slt", bufs=3) as spool, \
             tc.tile_pool(name="rcp", bufs=3) as rpool, \
             tc.tile_pool(name="gat", bufs=12) as gpool, \
             tc.tile_pool(name="one", bufs=12) as opool, \
             tc.tile_pool(name="res", bufs=4) as respool, \
             tc.tile_pool(name="pacc", bufs=2, space="PSUM") as pacc, \
             tc.tile_pool(name="pres", bufs=2, space="PSUM") as pres, \
             tc.tile_pool(name="dram", bufs=1, space="DRAM") as dpool:

            iota_t = cpool.tile([P, P], f32)
            nc.sync.dma_start(out=iota_t[:], in_=iota_in[:])
            wgt_t = cpool.tile([D, D], bf16)
            nc.sync.dma_start(out=wgt_t[:], in_=wgt[:])
            bias_t = cpool.tile([P, D], f32)
            nc.sync.dma_start(out=bias_t[:], in_=bias_bc[:])

            y_shard = dpool.tile([ESH, D], bf16)
            y_full = dpool.tile([NCORES * ESH, D], bf16)
            y_ch = [dpool.tile([NCORES * CH_E, D], bf16, addr_space="Shared",
                               name=f"y_ch{k}") for k in range(NCH)]

            # ---------------- Phase A ----------------
            for w in range(W_A):
                vidx_t = ipool.tile([P, FA], i32)
                nc.sync.dma_start(out=vidx_t[:], in_=a_vidx[w])
                slot_t = spool.tile([P, FA], f32)
                nc.sync.dma_start(out=slot_t[:], in_=a_slot[w])
                recip_t = rpool.tile([P, 1], f32)
                nc.sync.dma_start(out=recip_t[:], in_=a_recip[w, :, None])
                acc = pacc.tile([P, D], f32)
                for f in range(FA):
                    g = gpool.tile([P, D], bf16)
                    nc.gpsimd.indirect_dma_start(
                        out=g[:], out_offset=None, in_=inp[:],
                        in_offset=bass.IndirectOffsetOnAxis(
                            ap=vidx_t[:, f:f + 1], axis=0))
                    o = opool.tile([P, P], bf16)
                    nc.vector.tensor_tensor(
                        out=o[:], in0=iota_t[:],
                        in1=slot_t[:, f:f + 1].to_broadcast([P, P]), op=eq)
                    nc.tensor.matmul(acc[:], lhsT=o[:], rhs=g[:],
                                     start=(f == 0), stop=(f == FA - 1))
                yw = respool.tile([P, D], bf16, name="yw", tag="yw")
                nc.vector.tensor_tensor(
                    out=yw[:], in0=acc[:],
                    in1=recip_t[:, :1].to_broadcast([P, D]), op=mul)
                nc.sync.dma_start(out=y_shard[w * P:(w + 1) * P, :], in_=yw[:])
                if (w + 1) % (W_A // NCH) == 0:
                    k = w // (W_A // NCH)
                    nc.gpsimd.collective_compute(
                        "AllGather", mybir.AluOpType.bypass,
                        replica_groups=[list(range(NCORES))],
                        ins=[y_shard[k * CH_E:(k + 1) * CH_E, :]],
                        outs=[y_ch[k].opt()])
                    nc.sync.dma_start(
                        out=y_full[k * NCORES * CH_E:
                                   (k + 1) * NCORES * CH_E, :],
                        in_=y_ch[k][:])

            # ---------------- Phase B ----------------
            for w in range(W_B):
                eidx_t = ipool.tile([P, FB], i32, name="eidx_t", tag="idx_b")
                nc.sync.dma_start(out=eidx_t[:], in_=b_eidx[w])
                slot_t = spool.tile([P, FB], f32, name="slot_tb", tag="slt_b")
                nc.sync.dma_start(out=slot_t[:], in_=b_slot[w])
                recip_t = rpool.tile([P, 1], f32, name="recip_tb")
                nc.sync.dma_start(out=recip_t[:], in_=b_recip[w, :, None])
                acc2 = pacc.tile([P, D], f32, name="acc2")
                for f in range(FB):
                    g = gpool.tile([P, D], bf16, name="gb")
                    nc.gpsimd.indirect_dma_start(
                        out=g[:], out_offset=None, in_=y_full[:],
                        in_offset=bass.IndirectOffsetOnAxis(
                            ap=eidx_t[:, f:f + 1], axis=0))
                    o = opool.tile([P, P], bf16, name="ob")
                    nc.vector.tensor_tensor(
                        out=o[:], in0=iota_t[:],
                        in1=slot_t[:, f:f + 1].to_broadcast([P, P]), op=eq)
                    nc.tensor.matmul(acc2[:], lhsT=g[:], rhs=o[:],
                                     start=(f == 0), stop=(f == FB - 1))
                zt = respool.tile([P, D], bf16, name="zt", tag="zt")
                nc.vector.tensor_copy(out=zt[:], in_=acc2[:])
                res_p = pres.tile([P, D], f32, name="res_p")
                nc.tensor.matmul(res_p[:], lhsT=zt[:], rhs=wgt_t[:],
                                 start=True, stop=True)
                tmp = respool.tile([P, D], f32, name="tmpb")
                nc.vector.tensor_tensor(
                    out=tmp[:], in0=res_p[:],
                    in1=recip_t[:, :1].to_broadcast([P, D]), op=mul)
                res = respool.tile([P, D], f32, name="resb")
                nc.vector.tensor_tensor(out=res[:], in0=tmp[:], in1=bias_t[:],
                                        op=add)
                nc.sync.dma_start(out=out[w * P:(w + 1) * P, :], in_=res[:])

    nc.compile()
    return nc


def kernel(input, weight, bias, V, E, num_edges):
    global LAST_RESULTS
    inp = np.ascontiguousarray(np.asarray(input), dtype=np.float32)
    wgt = np.ascontiguousarray(np.asarray(weight), dtype=np.float32)
    b = np.asarray(bias).astype(np.float32)
    pp = _preprocess(V, E)

    if os.environ.get('KERNEL_EMULATE'):
        return _emulate(pp, inp, wgt, b)

    from concourse.bass_utils import run_bass_kernel_spmd

    key = (pp['FA'], pp['FB'])
    if key not in _PROG_CACHE:
        _PROG_CACHE[key] = _build_program(*key)
    nc = _PROG_CACHE[key]

    import ml_dtypes
    bf = ml_dtypes.bfloat16
    iota_np = np.tile(np.arange(P, dtype=np.float32), (P, 1))
    bias_bc = np.tile(b[None, :], (P, 1)).astype(np.float32)
    in_maps = []
    for c in range(NCORES):
        in_maps.append(dict(
            input=inp.astype(bf), wgt=wgt.astype(bf),
            bias_bc=bias_bc, iota_in=iota_np,
            a_vidx=pp['a_vidx'][c], a_slot=pp['a_slot'][c],
            a_recip=pp['a_recip'][c],
            b_eidx=pp['b_eidx'][c], b_slot=pp['b_slot'][c],
            b_recip=pp['b_recip'][c]))

    trace = bool(os.environ.get('KERNEL_TRACE'))
    res = run_bass_kernel_spmd(nc, in_maps, list(range(NCORES)), trace=trace)
    LAST_RESULTS = res
    rows = []
    for c in range(NCORES):
        n = min(NSH, N_NODE - c * NSH)
        rows.append(res.results[c]['out'][:n])
    return np.concatenate(rows, 0).astype(np.float32)


# revision 14
# speedup vs baseline: 11.7085x; 7.3515x over previous
"""HGNN conv on 8 trn2 cores.

out = D_v^-1 H D_e^-1 H^T input W + bias   (W commutes past the aggregations)

Phase A (edge-sharded): cores own contiguous 3200-edge ranges. Entries sorted
by E; per 128-edge window, entry tiles are gathered (indirect DMA on V) and
scatter-accumulated into PSUM via one-hot matmul; window * recip_e -> y_shard.
AllGather y_shard -> y_full [25600,128] on every core.
Phase B (node-sharded): cores own contiguous 6272-node ranges. Entries sorted
by V; per 128-node window, gather y_full rows by E, one-hot matmul with
swapped operands accumulates Z^T; then Z@W (* recip_v) + bias -> out rows.
"""
import os
import sys

for _p in ('/opt/trn_rl_repo', '/root/.axon_site/_ro/trn_rl_repo'):
    if os.path.isdir(_p) and _p not in sys.path:
        sys.path.insert(0, _p)

import numpy as np

P = 128
NCORES = 8
N_NODE = 50000
N_EDGE = 25000
D = 128
ESH = 3200            # edges per core shard (8*3200 = 25600 >= 25000)
NSH = 6272            # nodes per core shard (49*128; 8*6272 = 50176 >= 50000)
W_A = ESH // P        # 25 edge windows per core
W_B = NSH // P        # 49 node windows per core
NCH = 5               # allgather chunks (5 windows = 640 edges each)
CH_E = ESH // NCH     # 640 edges per chunk per core


def _row_of_edge(e):
    """y_full row for global edge id, chunk-major allgather layout."""
    c = e // ESH
    k = (e % ESH) // CH_E
    return k * (NCORES * CH_E) + c * CH_E + e % CH_E

_PROG_CACHE = {}
LAST_RESULTS = None


def _pack_windows(sorted_idx, sorted_slotbase, bnd, n_shards, n_win, F):
    """sorted_idx: gather row per entry; bnd: window entry boundaries."""
    vidx = np.zeros((n_shards, n_win, P, F), np.int32)
    slot = np.full((n_shards, n_win, P, F), -1.0, np.float32)
    for c in range(n_shards):
        for w in range(n_win):
            wi = c * n_win + w
            lo, hi = bnd[wi], bnd[wi + 1]
            n = hi - lo
            if n == 0:
                continue
            vv = np.zeros(F * P, np.int32)
            vv[:n] = sorted_idx[lo:hi]
            ss = np.full(F * P, -1.0, np.float32)
            ss[:n] = sorted_slotbase[lo:hi] - wi * P
            vidx[c, w] = vv.reshape(F, P).T
            slot[c, w] = ss.reshape(F, P).T
    return vidx, slot


def _preprocess(V, E):
    V = np.asarray(V).astype(np.int64)
    E = np.asarray(E).astype(np.int64)

    oA = np.argsort(E, kind='stable')
    Es, Vs = E[oA], V[oA]
    bndA = np.searchsorted(Es, np.arange(0, NCORES * ESH + 1, P))
    FA = int(np.ceil(np.diff(bndA).max() / P))
    a_vidx, a_slot = _pack_windows(Vs, Es, bndA, NCORES, W_A, FA)
    cntE = np.bincount(E, minlength=N_EDGE).astype(np.float64)
    recipE = (1.0 / np.maximum(cntE, 1.0)).astype(np.float32)
    er = np.arange(NCORES * ESH)
    a_recip = np.where(er < N_EDGE, recipE[np.minimum(er, N_EDGE - 1)],
                       0.0).astype(np.float32).reshape(NCORES, W_A, P)

    oB = np.argsort(V, kind='stable')
    Vs2, Es2 = V[oB], E[oB]
    bndB = np.searchsorted(Vs2, np.arange(0, NCORES * NSH + 1, P))
    FB = int(np.ceil(np.diff(bndB).max() / P))
    b_eidx, b_slot = _pack_windows(_row_of_edge(Es2), Vs2, bndB,
                                   NCORES, W_B, FB)
    cntV = np.bincount(V, minlength=N_NODE).astype(np.float64)
    recipV = (1.0 / np.maximum(cntV, 1.0)).astype(np.float32)
    nr = np.arange(NCORES * NSH)
    b_recip = np.where(nr < N_NODE, recipV[np.minimum(nr, N_NODE - 1)],
                       0.0).astype(np.float32).reshape(NCORES, W_B, P)

    return dict(FA=FA, FB=FB, a_vidx=a_vidx, a_slot=a_slot, a_recip=a_recip,
                b_eidx=b_eidx, b_slot=b_slot, b_recip=b_recip)


def _emulate(pp, inp_f32, weight, bias):
    """Numpy emulation of the exact device program (for logic validation)."""
    FA, FB = pp['FA'], pp['FB']
    iota = np.arange(P, dtype=np.float32)
    y_full = np.zeros((NCORES * ESH, D), np.float32)
    for c in range(NCORES):
        for w in range(W_A):
            acc = np.zeros((P, D), np.float32)
            for f in range(FA):
                g = inp_f32[pp['a_vidx'][c, w, :, f]]
                o = (iota[None, :] == pp['a_slot'][c, w, :, f][:, None])
                acc += o.astype(np.float32).T @ g
            r0 = _row_of_edge(c * ESH + w * P)
            y_full[r0:r0 + P] = acc * pp['a_recip'][c, w][:, None]
    out = np.zeros((NCORES * NSH, D), np.float32)
    for c in range(NCORES):
        for w in range(W_B):
            acc2 = np.zeros((D, P), np.float32)
            for f in range(FB):
                g = y_full[pp['b_eidx'][c, w, :, f]]
                o = (iota[None, :] == pp['b_slot'][c, w, :, f][:, None])
                acc2 += g.T @ o.astype(np.float32)
            z = acc2.T
            res = (z @ weight) * pp['b_recip'][c, w][:, None] + bias[None, :]
            out[(c * W_B + w) * P + c * (NSH - W_B * P):][:P] = res  # NSH==W_B*P
    rows = []
    for c in range(NCORES):
        n = min(NSH, N_NODE - c * NSH)
        rows.append(out[c * NSH:c * NSH + n])
    return np.concatenate(rows, 0)


def _build_program(FA, FB):
    import concourse.bacc as bacc
    import concourse.bass as bass
    import concourse.tile as tile
    from concourse import mybir

    f32 = mybir.dt.float32
    bf16 = mybir.dt.bfloat16
    i32 = mybir.dt.int32

    nc = bacc.Bacc(None, target_bir_lowering=False, debug=False)
    inp = nc.dram_tensor("input", [N_NODE, D], bf16, kind="ExternalInput")
    wgt = nc.dram_tensor("wgt", [D, D], bf16, kind="ExternalInput")
    bias_bc = nc.dram_tensor("bias_bc", [P, D], f32, kind="ExternalInput")
    iota_in = nc.dram_tensor("iota_in", [P, P], f32, kind="ExternalInput")
    a_vidx = nc.dram_tensor("a_vidx", [W_A, P, FA], i32, kind="ExternalInput")
    a_slot = nc.dram_tensor("a_slot", [W_A, P, FA], f32, kind="ExternalInput")
    a_recip = nc.dram_tensor("a_recip", [W_A, P], f32, kind="ExternalInput")
    b_eidx = nc.dram_tensor("b_eidx", [W_B, P, FB], i32, kind="ExternalInput")
    b_slot = nc.dram_tensor("b_slot", [W_B, P, FB], f32, kind="ExternalInput")
    b_recip = nc.dram_tensor("b_recip", [W_B, P], f32, kind="ExternalInput")
    out = nc.dram_tensor("out", [NSH, D], f32, kind="ExternalOutput")

    eq = mybir.AluOpType.is_equal
    mul = mybir.AluOpType.mult
    add = mybir.AluOpType.add

    with tile.TileContext(nc) as tc:
        with tc.tile_pool(name="const", bufs=1) as cpool, \
             tc.tile_pool(name="idx", bufs=3) as ipool, \
             tc.tile_pool(name="slt", bufs=3) as spool, \
             tc.tile_pool(name="rcp", bufs=3) as rpool, \
             tc.tile_pool(name="gat", bufs=3) as gpool, \
             tc.tile_pool(name="one", bufs=8) as opool, \
             tc.tile_pool(name="res", bufs=4) as respool, \
             tc.tile_pool(name="pacc", bufs=2, space="PSUM") as pacc, \
             tc.tile_pool(name="pres", bufs=2, space="PSUM") as pres, \
             tc.tile_pool(name="dram", bufs=1, space="DRAM") as dpool:

            iota_t = cpool.tile([P, P], f32)
            nc.sync.dma_start(out=iota_t[:], in_=iota_in[:])
            wgt_t = cpool.tile([D, D], bf16)
            nc.sync.dma_start(out=wgt_t[:], in_=wgt[:])
            bias_t = cpool.tile([P, D], f32)
            nc.sync.dma_start(out=bias_t[:], in_=bias_bc[:])

            y_shard = dpool.tile([ESH, D], bf16)
            y_full = dpool.tile([NCORES * ESH, D], bf16)
            y_ch = [dpool.tile([NCORES * CH_E, D], bf16, addr_space="Shared",
                               name=f"y_ch{k}") for k in range(NCH)]

            # ---------------- Phase A ----------------
            for w in range(W_A):
                vidx_t = ipool.tile([P, FA], i32)
                nc.sync.dma_start(out=vidx_t[:], in_=a_vidx[w])
                slot_t = spool.tile([P, FA], f32)
                nc.sync.dma_start(out=slot_t[:], in_=a_slot[w])
                recip_t = rpool.tile([P, 1], f32)
                nc.sync.dma_start(out=recip_t[:], in_=a_recip[w, :, None])
                acc = pacc.tile([P, D], f32)
                gw = gpool.tile([P, FA * D], bf16, name="gw", tag="gw_a")
                nc.gpsimd.indirect_dma_start(
                    out=gw[:], out_offset=None, in_=inp[:],
                    in_offset=bass.IndirectOffsetOnAxis(
                        ap=vidx_t[:, :FA], axis=0))
                for f in range(FA):
                    o = opool.tile([P, P], bf16)
                    nc.vector.tensor_tensor(
                        out=o[:], in0=iota_t[:],
                        in1=slot_t[:, f:f + 1].to_broadcast([P, P]), op=eq)
                    nc.tensor.matmul(acc[:], lhsT=o[:],
                                     rhs=gw[:, f * D:(f + 1) * D],
                                     start=(f == 0), stop=(f == FA - 1))
                yw = respool.tile([P, D], bf16, name="yw", tag="yw")
                nc.vector.tensor_tensor(
                    out=yw[:], in0=acc[:],
                    in1=recip_t[:, :1].to_broadcast([P, D]), op=mul)
                nc.sync.dma_start(out=y_shard[w * P:(w + 1) * P, :], in_=yw[:])
                if (w + 1) % (W_A // NCH) == 0:
                    k = w // (W_A // NCH)
                    nc.gpsimd.collective_compute(
                        "AllGather", mybir.AluOpType.bypass,
                        replica_groups=[list(range(NCORES))],
                        ins=[y_shard[k * CH_E:(k + 1) * CH_E, :]],
                        outs=[y_ch[k].opt()])
                    nc.sync.dma_start(
                        out=y_full[k * NCORES * CH_E:
                                   (k + 1) * NCORES * CH_E, :],
                        in_=y_ch[k][:])

            # ---------------- Phase B ----------------
            for w in range(W_B):
                eidx_t = ipool.tile([P, FB], i32, name="eidx_t", tag="idx_b")
                nc.sync.dma_start(out=eidx_t[:], in_=b_eidx[w])
                slot_t = spool.tile([P, FB], f32, name="slot_tb", tag="slt_b")
                nc.sync.dma_start(out=slot_t[:], in_=b_slot[w])
                recip_t = rpool.tile([P, 1], f32, name="recip_tb")
                nc.sync.dma_start(out=recip_t[:], in_=b_recip[w, :, None])
                acc2 = pacc.tile([P, D], f32, name="acc2")
                gwb = gpool.tile([P, FB * D], bf16, name="gwb", tag="gw_b")
                nc.gpsimd.indirect_dma_start(
                    out=gwb[:], out_offset=None, in_=y_full[:],
                    in_offset=bass.IndirectOffsetOnAxis(
                        ap=eidx_t[:, :FB], axis=0))
                for f in range(FB):
                    o = opool.tile([P, P], bf16, name="ob")
                    nc.vector.tensor_tensor(
                        out=o[:], in0=iota_t[:],
                        in1=slot_t[:, f:f + 1].to_broadcast([P, P]), op=eq)
                    nc.tensor.matmul(acc2[:], lhsT=gwb[:, f * D:(f + 1) * D],
                                     rhs=o[:],
                                     start=(f == 0), stop=(f == FB - 1))
                zt = respool.tile([P, D], bf16, name="zt", tag="zt")
                nc.vector.tensor_copy(out=zt[:], in_=acc2[:])
                res_p = pres.tile([P, D], f32, name="res_p")
                nc.tensor.matmul(res_p[:], lhsT=zt[:], rhs=wgt_t[:],
                                 start=True, stop=True)
                tmp = respool.tile([P, D], f32, name="tmpb")
                nc.vector.tensor_tensor(
                    out=tmp[:], in0=res_p[:],
                    in1=recip_t[:, :1].to_broadcast([P, D]), op=mul)
                res = respool.tile([P, D], f32, name="resb")
                nc.vector.tensor_tensor(out=res[:], in0=tmp[:], in1=bias_t[:],
                                        op=add)
                nc.sync.dma_start(out=out[w * P:(w + 1) * P, :], in_=res[:])

    nc.compile()
    return nc


def kernel(input, weight, bias, V, E, num_edges):
    global LAST_RESULTS
    inp = np.ascontiguousarray(np.asarray(input), dtype=np.float32)
    wgt = np.ascontiguousarray(np.asarray(weight), dtype=np.float32)
    b = np.asarray(bias).astype(np.float32)
    pp = _preprocess(V, E)

    if os.environ.get('KERNEL_EMULATE'):
        return _emulate(pp, inp, wgt, b)

    from concourse.bass_utils import run_bass_kernel_spmd

    key = (pp['FA'], pp['FB'])
    if key not in _PROG_CACHE:
        _PROG_CACHE[key] = _build_program(*key)
    nc = _PROG_CACHE[key]

    import ml_dtypes
    bf = ml_dtypes.bfloat16
    iota_np = np.tile(np.arange(P, dtype=np.float32), (P, 1))
    bias_bc = np.tile(b[None, :], (P, 1)).astype(np.float32)
    in_maps = []
    for c in range(NCORES):
        in_maps.append(dict(
            input=inp.astype(bf), wgt=wgt.astype(bf),
            bias_bc=bias_bc, iota_in=iota_np,
            a_vidx=pp['a_vidx'][c], a_slot=pp['a_slot'][c],
            a_recip=pp['a_recip'][c],
            b_eidx=pp['b_eidx'][c], b_slot=pp['b_slot'][c],
            b_recip=pp['b_recip'][c]))

    trace = bool(os.environ.get('KERNEL_TRACE'))
    res = run_bass_kernel_spmd(nc, in_maps, list(range(NCORES)), trace=trace)
    LAST_RESULTS = res
    rows = []
    for c in range(NCORES):
        n = min(NSH, N_NODE - c * NSH)
        rows.append(res.results[c]['out'][:n])
    return np.concatenate(rows, 0).astype(np.float32)
